# revision 1
# baseline (speedup 1.0000x reference)
"""Trainium2 Bass kernel for nn_DecodeSBP (keypoint heatmap decode).

Contract: kernel(x=[1,133,512,512] f32) -> [133,3] f32
  joints[k] = (4*xx, 4*yy, conf) if conf > 0.8 else (-4, -4, -1)
  where flat = argmax(sigmoid(x[0,k])), conf = sigmoid(max), yy = flat//512,
  xx = flat%512. sigmoid is monotonic so the argmax runs on raw logits.

Sharding: keypoint dim across 8 cores (17/core, core 7 zero-padded).

Per-core program (one full-data pass, hierarchical argmax):
  phase 1: stream 17 MB; one DVE reduce_max pass emits per-(partition,
    512-chunk) maxes pmax4[128, 4 per keypoint].
  finale (two halves; half 1 hides under streaming of half 2):
    TensorE-transpose the 4 chunk columns into one PSUM tile [kh, 512]
    whose column c*128+p ranks chunk (p, c); global max per keypoint;
    "mask * reversed-rank-iota + reduce_max" argmax -> winning chunk;
    gather each winner's 2 KB chunk from DRAM via register-offset DMAs;
    same idiom for the index inside the chunk; integer decode to
    (4*xx, 4*yy, conf) with a confidence-gated copy_predicated.
"""

import sys
from contextlib import ExitStack

for _p in ("/opt/trn_rl_repo", "/opt/pypackages"):
    if _p not in sys.path:
        sys.path.append(_p)

import numpy as np

import concourse.bacc as bacc
import concourse.bass as bass
import concourse.tile as tile
from concourse import mybir
from concourse.bass_utils import run_bass_kernel_spmd
from concourse.masks import make_identity

K = 17          # keypoints per core
NK = 133        # total keypoints
ROW = 262144    # 512*512
P = 128         # SBUF partitions
F = ROW // P    # 2048 free elems per partition
C = 4           # chunks per partition row
S = F // C      # 512 elems per chunk
W = 512
N_CORES = 8
KSPLIT = 9      # finale half 1 covers keypoints [0, KSPLIT)
TILES = (1, 2, 3, 3, 3, 3, 2)   # keypoints per stream tile (sum 17)

f32 = mybir.dt.float32
i32 = mybir.dt.int32
Alu = mybir.AluOpType
Act = mybir.ActivationFunctionType

_NC_CACHE = None


def _build():
    nc = bacc.Bacc("TRN2", target_bir_lowering=False, debug=False)
    x_dram = nc.dram_tensor("x", [K, ROW], f32, kind="ExternalInput")
    out_dram = nc.dram_tensor("out", [K, 3], f32, kind="ExternalOutput")

    x_pkf = x_dram.ap().rearrange("k (p f) -> p k f", f=F)      # [128, K, 2048]
    x_flat = x_dram.ap().rearrange("k f -> (k f)")

    with tile.TileContext(nc) as tc, ExitStack() as ctx:
        const_pool = ctx.enter_context(tc.tile_pool(name="const", bufs=1))
        in_pool = ctx.enter_context(
            tc.tile_pool(name="in", bufs=len(TILES)))
        small_pool = ctx.enter_context(tc.tile_pool(name="small", bufs=1))
        psum_pool = ctx.enter_context(
            tc.tile_pool(name="psum", bufs=1, space="PSUM"))

        ident = const_pool.tile([P, P], f32)
        make_identity(nc, ident[:])
        # riota_pc[k, c*128+p] = 512 - (4p + c): rank of chunk (p,c) in flat
        # order, reversed so reduce_max picks the first occurrence.
        riota_pc = const_pool.tile([K, C * P], f32)
        nc.gpsimd.iota(riota_pc[:].rearrange("k (c p) -> k c p", p=P),
                       pattern=[[-1, C], [-C, P]], base=C * P,
                       channel_multiplier=0,
                       allow_small_or_imprecise_dtypes=True)
        # riota_j[k, j] = 512 - j
        riota_j = const_pool.tile([K, S], f32)
        nc.gpsimd.iota(riota_j[:], pattern=[[-1, S]], base=S,
                       channel_multiplier=0,
                       allow_small_or_imprecise_dtypes=True)

        # per-(partition, chunk) maxes, split per finale half
        pmax_a = small_pool.tile([P, KSPLIT * C], f32)
        pmax_b = small_pool.tile([P, (K - KSPLIT) * C], f32)

        def stream(k_lo, k_hi, tiles):
            k0 = k_lo
            for g in tiles:
                t = in_pool.tile([P, g * F], f32, tag="xin")
                nc.sync.dma_start(
                    t[:].rearrange("p (g f) -> p g f", f=F),
                    x_pkf[:, k0:k0 + g, :])
                pm, ofs = (pmax_a, 0) if k_lo < KSPLIT else (pmax_b, KSPLIT)
                nc.vector.reduce_max(
                    pm[:, (k0 - ofs) * C:(k0 - ofs + g) * C],
                    t[:].rearrange("p (g c s) -> p g c s", c=C, s=S),
                    axis=mybir.AxisListType.X)
                k0 += g
            assert k0 == k_hi

        def finale(h, k_lo, k_hi, engines):
            kh = k_hi - k_lo
            pm = pmax_a if h == 0 else pmax_b
            pm3 = pm[:].rearrange("p (k c) -> p k c", c=C)

            # transpose chunk columns -> psumT[k, c*128+p] = chunkmax(p, c)
            psumT = psum_pool.tile([kh, C * P], f32, tag=f"psumT{h}")
            for c in range(C):
                nc.tensor.matmul(psumT[:, c * P:(c + 1) * P],
                                 pm3[:, :, c], ident[:], is_transpose=True)

            gmax = small_pool.tile([kh, 1], f32, tag=f"gmax{h}")
            nc.vector.reduce_max(gmax[:], psumT[:], axis=mybir.AxisListType.X)
            cand_p = small_pool.tile([kh, C * P], f32, tag=f"cand_p{h}")
            nc.vector.scalar_tensor_tensor(
                cand_p[:], in0=psumT[:], scalar=gmax[:],
                in1=riota_pc[0:kh, :], op0=Alu.is_ge, op1=Alu.mult)
            rc = small_pool.tile([kh, 1], f32, tag=f"rc{h}")  # 512 - chunkrank
            nc.vector.reduce_max(rc[:], cand_p[:], axis=mybir.AxisListType.X)

            # DRAM element offset of winning chunk, per keypoint partition:
            # offs = (512*(k+1) - rc) * 512 = 262144*(k+1) - 512*rc
            kiota = const_pool.tile([kh, 1], f32, tag=f"kiota{h}")
            nc.gpsimd.iota(kiota[:], pattern=[[0, 1]], base=ROW * (k_lo + 1),
                           channel_multiplier=ROW,
                           allow_small_or_imprecise_dtypes=True)
            offs_i = small_pool.tile([kh, 1], i32, tag=f"offs_i{h}")
            nc.vector.scalar_tensor_tensor(
                offs_i[:], in0=rc[:], scalar=-float(W), in1=kiota[:],
                op0=Alu.mult, op1=Alu.add)

            grow = small_pool.tile([kh, S], f32, tag=f"grow{h}")
            for k in range(kh):
                eng = engines[k % len(engines)]
                reg = eng.alloc_register()
                eng.load(reg, offs_i[k:k + 1, 0:1])
                off = eng.snap(reg, donate=True)
                eng.dma_start(grow[k:k + 1, :], x_flat[bass.ds(off, S)])

            # index within the winning chunk
            cand_j = small_pool.tile([kh, S], f32, tag=f"cand_j{h}")
            nc.vector.scalar_tensor_tensor(
                cand_j[:], in0=grow[:], scalar=gmax[:], in1=riota_j[0:kh, :],
                op0=Alu.is_ge, op1=Alu.mult)
            rj = small_pool.tile([kh, 1], f32, tag=f"rj{h}")  # 512 - j*
            nc.vector.reduce_max(rj[:], cand_j[:], axis=mybir.AxisListType.X)

            # flat = chunkrank*512 + j* = 262656 - 512*rc - rj
            flat = small_pool.tile([kh, 1], f32, tag=f"flat{h}")
            nc.vector.scalar_tensor_tensor(
                flat[:], in0=rc[:], scalar=float(W), in1=rj[:],
                op0=Alu.mult, op1=Alu.add)
            flat_i = small_pool.tile([kh, 1], i32, tag=f"flat_i{h}")
            nc.vector.tensor_scalar(flat_i[:], flat[:], -1.0,
                                    float(C * P * W + W), Alu.mult, Alu.add)
            xx_i = small_pool.tile([kh, 1], i32, tag=f"xx_i{h}")
            nc.vector.tensor_scalar(xx_i[:], flat_i[:], W - 1, None,
                                    Alu.bitwise_and)
            yy_i = small_pool.tile([kh, 1], i32, tag=f"yy_i{h}")
            nc.vector.tensor_scalar(yy_i[:], flat_i[:], 9, None,
                                    Alu.logical_shift_right)

            conf = small_pool.tile([kh, 1], f32, tag=f"conf{h}")
            nc.scalar.activation(conf[:], gmax[:], Act.Sigmoid)
            valid = small_pool.tile([kh, 1], f32, tag=f"valid{h}")
            nc.vector.tensor_scalar(valid[:], conf[:], 0.8, None, Alu.is_gt)

            # out = valid ? (4xx, 4yy, conf) : (-4, -4, -1)
            cand = small_pool.tile([kh, 3], f32, tag=f"cand{h}")
            nc.vector.tensor_scalar(cand[:, 0:1], xx_i[:], 4.0, None, Alu.mult)
            nc.vector.tensor_scalar(cand[:, 1:2], yy_i[:], 4.0, None, Alu.mult)
            nc.vector.tensor_copy(cand[:, 2:3], conf[:])
            vb3 = small_pool.tile([kh, 3], i32, tag=f"vb3{h}")
            nc.vector.tensor_scalar(vb3[:], cand[:], 0.0, valid[:],
                                    Alu.mult, Alu.add)
            out_sb = small_pool.tile([kh, 3], f32, tag=f"out_sb{h}")
            nc.vector.memset(out_sb[:, 0:2], -4.0)
            nc.vector.memset(out_sb[:, 2:3], -1.0)
            nc.vector.copy_predicated(out_sb[:], vb3[:], cand[:])
            nc.scalar.dma_start(out_dram.ap()[k_lo:k_hi, :], out_sb[:])

        stream(0, KSPLIT, TILES[:4])
        # half-1 gather on gpsimd (SWDGE) so it doesn't queue behind half-2
        # streaming on the sync HWDGE ring.
        finale(0, 0, KSPLIT, [nc.gpsimd])
        stream(KSPLIT, K, TILES[4:])
        finale(1, KSPLIT, K, [nc.sync, nc.scalar])

    nc.compile()
    return nc


def _get_nc():
    global _NC_CACHE
    if _NC_CACHE is None:
        _NC_CACHE = _build()
    return _NC_CACHE


def _shard(x: np.ndarray) -> list[dict[str, np.ndarray]]:
    xf = np.ascontiguousarray(np.asarray(x, dtype=np.float32).reshape(NK, ROW))
    shards = []
    for c in range(N_CORES):
        lo = c * K
        s = xf[lo:min(lo + K, NK)]
        if s.shape[0] < K:
            s = np.concatenate(
                [s, np.zeros((K - s.shape[0], ROW), np.float32)], axis=0)
        shards.append({"x": np.ascontiguousarray(s)})
    return shards


def _run(x, trace=False, **kw):
    nc = _get_nc()
    res = run_bass_kernel_spmd(nc, _shard(x), core_ids=list(range(N_CORES)),
                               trace=trace, **kw)
    out = np.concatenate([r["out"] for r in res.results], axis=0)[:NK]
    return out.astype(np.float32), res


def kernel(x: np.ndarray) -> np.ndarray:
    out, _ = _run(x, trace=False)
    return out



# revision 13
# speedup vs baseline: 1.1336x; 1.1336x over previous
"""Trainium2 Bass kernel for nn_DecodeSBP (keypoint heatmap decode).

Contract: kernel(x=[1,133,512,512] f32) -> [133,3] f32
  joints[k] = (4*xx, 4*yy, conf) if conf > 0.8 else (-4, -4, -1)
  where flat = argmax(sigmoid(x[0,k])), conf = sigmoid(max), yy = flat//512,
  xx = flat%512. sigmoid is monotonic so the argmax runs on raw logits.

Sharding: keypoint dim across 8 cores (17/core, core 7 zero-padded).

Per-core program (one full-data pass, hierarchical argmax), v2:
  stream: 7 tiles (2,3,3,3,3,2,1 keypoints). Each keypoint row [128,2048]
    is two 1024-chunks; a fused tensor_tensor_reduce per chunk computes
    max(first 512-half, second 512-half) and reduces it in the same pass,
    so DVE touches each element only half as often as a plain reduce.
  finales: 4 groups (5,6,3,3 keypoints) aligned to tile boundaries;
    each group's chain (PE transpose -> global max -> rank-trick chunk
    argmax -> register-offset gather of the winning 4KB chunk -> in-chunk
    argmax -> integer decode) runs while later tiles stream. Gathers are
    spread over the scalar/sync/gpsimd sequencers to parallelize the
    ~1.3us register round-trips. Only the 3-keypoint last group's chain
    sits in the tail, fed by a 1-keypoint final tile.
"""

import sys
from contextlib import ExitStack

for _p in ("/opt/trn_rl_repo", "/opt/pypackages"):
    if _p not in sys.path:
        sys.path.append(_p)

import numpy as np

import concourse.bacc as bacc
import concourse.bass as bass
import concourse.tile as tile
from concourse import mybir
from concourse.bass_utils import run_bass_kernel_spmd
from concourse.masks import make_identity

K = 17          # keypoints per core
NK = 133        # total keypoints
ROW = 262144    # 512*512
P = 128         # SBUF partitions
F = ROW // P    # 2048 free elems per partition
C = 2           # chunks per partition row
S = F // C      # 1024 elems per chunk
H = S // 2      # 512: ttr half-chunk
W = 512
N_CORES = 8
TILES = (2, 3, 3, 3, 3, 2, 1)           # keypoints per stream tile (sum 17)
GROUPS = ((0, 5), (5, 11), (11, 14), (14, 17))  # finale groups [k_lo, k_hi)
MAXG = max(hi - lo for lo, hi in GROUPS)

f32 = mybir.dt.float32
i32 = mybir.dt.int32
Alu = mybir.AluOpType
Act = mybir.ActivationFunctionType

USE_TTR = False

_NC_CACHE = None


def _build():
    nc = bacc.Bacc("TRN2", target_bir_lowering=False, debug=False)
    x_dram = nc.dram_tensor("x", [K, ROW], f32, kind="ExternalInput")
    out_dram = nc.dram_tensor("out", [K, 3], f32, kind="ExternalOutput")

    x_pkf = x_dram.ap().rearrange("k (p f) -> p k f", f=F)      # [128, K, 2048]
    x_flat = x_dram.ap().rearrange("k f -> (k f)")

    with tile.TileContext(nc) as tc, ExitStack() as ctx:
        const_pool = ctx.enter_context(tc.tile_pool(name="const", bufs=1))
        in_pool = ctx.enter_context(
            tc.tile_pool(name="in", bufs=len(TILES) - 1))
        small_pool = ctx.enter_context(tc.tile_pool(name="small", bufs=1))
        psum_pool = ctx.enter_context(
            tc.tile_pool(name="psum", bufs=2, space="PSUM"))

        ident = const_pool.tile([P, P], f32)
        make_identity(nc, ident[:])
        # riota_pc[k, c*128+p] = 256 - (2p + c): rank of chunk (p,c) in flat
        # order, reversed so reduce_max picks the first occurrence.
        riota_pc = const_pool.tile([MAXG, C * P], f32)
        nc.gpsimd.iota(riota_pc[:].rearrange("k (c p) -> k c p", p=P),
                       pattern=[[-1, C], [-C, P]], base=C * P,
                       channel_multiplier=0,
                       allow_small_or_imprecise_dtypes=True)
        # riota_j[k, j] = 1024 - j
        riota_j = const_pool.tile([MAXG, S], f32)
        nc.gpsimd.iota(riota_j[:], pattern=[[-1, S]], base=S,
                       channel_multiplier=0,
                       allow_small_or_imprecise_dtypes=True)
        # kiota[g][i, 0] = ROW * (k_lo + i + 1)
        kiota = {}
        for gi, (k_lo, k_hi) in enumerate(GROUPS):
            kio = const_pool.tile([k_hi - k_lo, 1], f32, tag=f"kiota{gi}")
            nc.gpsimd.iota(kio[:], pattern=[[0, 1]], base=ROW * (k_lo + 1),
                           channel_multiplier=ROW,
                           allow_small_or_imprecise_dtypes=True)
            kiota[gi] = kio

        # ttr elementwise-max byproduct, write-only scratch (qr.py idiom:
        # broadcast a [P,1] tile over the input shape to discard it)
        scratch = const_pool.tile([P, 1], f32)

        # per-(partition, chunk) maxes, one tile per finale group
        pmax = {}
        for gi, (lo, hi) in enumerate(GROUPS):
            pm_t = small_pool.tile([P, (hi - lo) * C], f32, tag=f"pmax{gi}")
            pmax[gi] = pm_t

        def group_of(k):
            for gi, (lo, hi) in enumerate(GROUPS):
                if lo <= k < hi:
                    return gi
            raise AssertionError(k)

        def stream_tile(k0, g):
            t = in_pool.tile([P, g * F], f32, tag="xin")
            nc.sync.dma_start(
                t[:].rearrange("p (g f) -> p g f", f=F),
                x_pkf[:, k0:k0 + g, :])
            gi = group_of(k0)
            assert group_of(k0 + g - 1) == gi, "tile spans groups"
            k_lo = GROUPS[gi][0]
            o = k0 - k_lo
            if USE_TTR:
                # Fused max-combine + reduce: one ttr per 1024-chunk reads
                # the two 512-halves once and emits the chunk max.
                for j in range(g):
                    for c in range(C):
                        base = j * F + c * S
                        nc.vector.tensor_tensor_reduce(
                            out=scratch[:].broadcast_to((P, H)),
                            in0=t[:, base:base + H],
                            in1=t[:, base + H:base + S],
                            scale=1.0, scalar=-1.0e30,
                            op0=Alu.max, op1=Alu.max,
                            accum_out=pmax[gi][:, (o + j) * C + c:
                                               (o + j) * C + c + 1])
            else:
                nc.vector.reduce_max(
                    pmax[gi][:, o * C:(o + g) * C],
                    t[:].rearrange("p (q s) -> p q s", s=S),
                    axis=mybir.AxisListType.X)

        # finale state per group
        st = {}

        def finale_pre(gi, gather_engines):
            k_lo, k_hi = GROUPS[gi]
            kh = k_hi - k_lo
            pm3 = pmax[gi][:].rearrange("p (k c) -> p k c", c=C)

            # transpose chunk columns -> psumT[k, c*128+p] = chunkmax(p, c)
            psumT = psum_pool.tile([kh, C * P], f32, tag=f"psumT{gi % 2}")
            for c in range(C):
                nc.tensor.matmul(psumT[:, c * P:(c + 1) * P],
                                 pm3[:, :, c], ident[:], is_transpose=True)

            gmax = small_pool.tile([kh, 1], f32, tag=f"gmax{gi}")
            nc.vector.reduce_max(gmax[:], psumT[:], axis=mybir.AxisListType.X)
            cand_p = small_pool.tile([kh, C * P], f32, tag=f"cand_p{gi}")
            nc.vector.scalar_tensor_tensor(
                cand_p[:], in0=psumT[:], scalar=gmax[:],
                in1=riota_pc[0:kh, :], op0=Alu.is_ge, op1=Alu.mult)
            rc = small_pool.tile([kh, 1], f32, tag=f"rc{gi}")  # 256 - chunkrank
            nc.vector.reduce_max(rc[:], cand_p[:], axis=mybir.AxisListType.X)

            # DRAM element offset of winning chunk, per keypoint partition:
            # offs = (256*(k+1) - rc) * 1024 = 262144*(k+1) - 1024*rc
            offs_i = small_pool.tile([kh, 1], i32, tag=f"offs_i{gi}")
            nc.vector.scalar_tensor_tensor(
                offs_i[:], in0=rc[:], scalar=-float(S), in1=kiota[gi][:],
                op0=Alu.mult, op1=Alu.add)

            grow = small_pool.tile([kh, S], f32, tag=f"grow{gi}")
            for k in range(kh):
                eng = gather_engines[k % len(gather_engines)]
                reg = eng.alloc_register()
                eng.load(reg, offs_i[k:k + 1, 0:1])
                off = eng.snap(reg, donate=True)
                eng.dma_start(grow[k:k + 1, :], x_flat[bass.ds(off, S)])

            conf = small_pool.tile([kh, 1], f32, tag=f"conf{gi}")
            nc.scalar.activation(conf[:], gmax[:], Act.Sigmoid)
            st[gi] = (gmax, rc, grow, conf)

        def finale_post(gi):
            k_lo, k_hi = GROUPS[gi]
            kh = k_hi - k_lo
            gmax, rc, grow, conf = st[gi]

            # index within the winning chunk
            cand_j = small_pool.tile([kh, S], f32, tag=f"cand_j{gi}")
            nc.vector.scalar_tensor_tensor(
                cand_j[:], in0=grow[:], scalar=gmax[:], in1=riota_j[0:kh, :],
                op0=Alu.is_ge, op1=Alu.mult)
            rj = small_pool.tile([kh, 1], f32, tag=f"rj{gi}")  # 1024 - j*
            nc.vector.reduce_max(rj[:], cand_j[:], axis=mybir.AxisListType.X)

            # flat = chunkrank*1024 + j* = 263168 - 1024*rc - rj
            flat = small_pool.tile([kh, 1], f32, tag=f"flat{gi}")
            nc.vector.scalar_tensor_tensor(
                flat[:], in0=rc[:], scalar=float(S), in1=rj[:],
                op0=Alu.mult, op1=Alu.add)
            flat_i = small_pool.tile([kh, 1], i32, tag=f"flat_i{gi}")
            nc.vector.tensor_scalar(flat_i[:], flat[:], -1.0,
                                    float(C * P * S + S), Alu.mult, Alu.add)
            xx_i = small_pool.tile([kh, 1], i32, tag=f"xx_i{gi}")
            nc.vector.tensor_scalar(xx_i[:], flat_i[:], W - 1, None,
                                    Alu.bitwise_and)
            yy_i = small_pool.tile([kh, 1], i32, tag=f"yy_i{gi}")
            nc.vector.tensor_scalar(yy_i[:], flat_i[:], 9, None,
                                    Alu.logical_shift_right)

            valid = small_pool.tile([kh, 1], f32, tag=f"valid{gi}")
            nc.vector.tensor_scalar(valid[:], conf[:], 0.8, None, Alu.is_gt)

            # out = valid ? (4xx, 4yy, conf) : (-4, -4, -1)
            cand = small_pool.tile([kh, 3], f32, tag=f"cand{gi}")
            nc.vector.tensor_scalar(cand[:, 0:1], xx_i[:], 4.0, None, Alu.mult)
            nc.vector.tensor_scalar(cand[:, 1:2], yy_i[:], 4.0, None, Alu.mult)
            nc.vector.tensor_copy(cand[:, 2:3], conf[:])
            vb3 = small_pool.tile([kh, 3], i32, tag=f"vb3{gi}")
            nc.vector.tensor_scalar(vb3[:], cand[:], 0.0, valid[:],
                                    Alu.mult, Alu.add)
            out_sb = small_pool.tile([kh, 3], f32, tag=f"out_sb{gi}")
            nc.vector.memset(out_sb[:, 0:2], -4.0)
            nc.vector.memset(out_sb[:, 2:3], -1.0)
            nc.vector.copy_predicated(out_sb[:], vb3[:], cand[:])
            nc.scalar.dma_start(out_dram.ap()[k_lo:k_hi, :], out_sb[:])

        big = [nc.scalar, nc.sync, nc.gpsimd]
        small = [nc.scalar, nc.sync, nc.gpsimd]

        # Emission (= per-engine program) order interleaves tiles and
        # finale halves so every chain hides under later streaming.
        cum = [0]
        for g in TILES:
            cum.append(cum[-1] + g)
        stream_tile(cum[0], TILES[0])           # k0-1
        stream_tile(cum[1], TILES[1])           # k2-4
        finale_pre(0, big)
        stream_tile(cum[2], TILES[2])           # k5-7
        finale_post(0)
        stream_tile(cum[3], TILES[3])           # k8-10
        finale_pre(1, big)
        stream_tile(cum[4], TILES[4])           # k11-13
        finale_post(1)
        finale_pre(2, small)
        stream_tile(cum[5], TILES[5])           # k14-15
        finale_post(2)
        stream_tile(cum[6], TILES[6])           # k16
        finale_pre(3, small)
        finale_post(3)

    nc.compile()
    return nc


def _get_nc():
    global _NC_CACHE
    if _NC_CACHE is None:
        _NC_CACHE = _build()
    return _NC_CACHE


def _shard(x: np.ndarray) -> list[dict[str, np.ndarray]]:
    xf = np.ascontiguousarray(np.asarray(x, dtype=np.float32).reshape(NK, ROW))
    shards = []
    for c in range(N_CORES):
        lo = c * K
        s = xf[lo:min(lo + K, NK)]
        if s.shape[0] < K:
            s = np.concatenate(
                [s, np.zeros((K - s.shape[0], ROW), np.float32)], axis=0)
        shards.append({"x": np.ascontiguousarray(s)})
    return shards


def _run(x, trace=False, **kw):
    nc = _get_nc()
    res = run_bass_kernel_spmd(nc, _shard(x), core_ids=list(range(N_CORES)),
                               trace=trace, **kw)
    out = np.concatenate([r["out"] for r in res.results], axis=0)[:NK]
    return out.astype(np.float32), res


def kernel(x: np.ndarray) -> np.ndarray:
    out, _ = _run(x, trace=False)
    return out


# revision 14
# speedup vs baseline: 1.1798x; 1.0408x over previous
"""Trainium2 Bass kernel for nn_DecodeSBP (keypoint heatmap decode).

Contract: kernel(x=[1,133,512,512] f32) -> [133,3] f32
  joints[k] = (4*xx, 4*yy, conf) if conf > 0.8 else (-4, -4, -1)
  where flat = argmax(sigmoid(x[0,k])), conf = sigmoid(max), yy = flat//512,
  xx = flat%512. sigmoid is monotonic so the argmax runs on raw logits.

Sharding: keypoint dim across 8 cores (17/core, core 7 zero-padded).

Per-core program (one full-data pass, hierarchical argmax), v3:
  All 7 stream-tile DMAs are issued upfront on the sync ring (nothing can
  delay them); the DVE chunk-max reduces and the per-group finale chains
  are interleaved in an order chosen so Vector never waits on a gather
  or transpose. Finale groups (5,6,5,1 keypoints): PE-transpose ->
  global max -> rank-trick chunk argmax -> register-offset gather of the
  winning 4KB chunk (scalar+sync sequencers only; gpsimd SWDGE completion
  sems measured ~6us late) -> in-chunk argmax (max_index) -> integer
  decode. The last group is the single last-streamed keypoint, so the
  post-stream tail is one short chain.
"""

import sys
from contextlib import ExitStack

for _p in ("/opt/trn_rl_repo", "/opt/pypackages"):
    if _p not in sys.path:
        sys.path.append(_p)

import numpy as np

import concourse.bacc as bacc
import concourse.bass as bass
import concourse.tile as tile
from concourse import mybir
from concourse.bass_utils import run_bass_kernel_spmd
from concourse.masks import make_identity

K = 17          # keypoints per core
NK = 133        # total keypoints
ROW = 262144    # 512*512
P = 128         # SBUF partitions
F = ROW // P    # 2048 free elems per partition
C = 2           # chunks per partition row
S = F // C      # 1024 elems per chunk
W = 512
N_CORES = 8
TILES = (2, 3, 3, 3, 3, 2, 1)           # keypoints per stream tile (sum 17)
GROUPS = ((0, 5), (5, 11), (11, 16), (16, 17))  # finale groups [k_lo, k_hi)
MAXG = max(hi - lo for lo, hi in GROUPS)

f32 = mybir.dt.float32
i32 = mybir.dt.int32
u32 = mybir.dt.uint32
Alu = mybir.AluOpType
Act = mybir.ActivationFunctionType

USE_MAXIDX = True

_NC_CACHE = None


def _build():
    nc = bacc.Bacc("TRN2", target_bir_lowering=False, debug=False)
    x_dram = nc.dram_tensor("x", [K, ROW], f32, kind="ExternalInput")
    out_dram = nc.dram_tensor("out", [K, 3], f32, kind="ExternalOutput")

    x_pkf = x_dram.ap().rearrange("k (p f) -> p k f", f=F)      # [128, K, 2048]
    x_flat = x_dram.ap().rearrange("k f -> (k f)")

    with tile.TileContext(nc) as tc, ExitStack() as ctx:
        const_pool = ctx.enter_context(tc.tile_pool(name="const", bufs=1))
        in_pool = ctx.enter_context(
            tc.tile_pool(name="in", bufs=len(TILES) - 1))
        small_pool = ctx.enter_context(tc.tile_pool(name="small", bufs=1))
        psum_pool = ctx.enter_context(
            tc.tile_pool(name="psum", bufs=2, space="PSUM"))

        ident = const_pool.tile([P, P], f32)
        make_identity(nc, ident[:])
        # riota_pc[k, c*128+p] = 256 - (2p + c): rank of chunk (p,c) in flat
        # order, reversed so reduce_max picks the first occurrence.
        riota_pc = const_pool.tile([MAXG, C * P], f32)
        nc.gpsimd.iota(riota_pc[:].rearrange("k (c p) -> k c p", p=P),
                       pattern=[[-1, C], [-C, P]], base=C * P,
                       channel_multiplier=0,
                       allow_small_or_imprecise_dtypes=True)
        # riota_j[k, j] = 1024 - j  (rank-trick fallback when not USE_MAXIDX)
        riota_j = const_pool.tile([MAXG, S], f32)
        nc.gpsimd.iota(riota_j[:], pattern=[[-1, S]], base=S,
                       channel_multiplier=0,
                       allow_small_or_imprecise_dtypes=True)
        # kiota[g][i, 0] = ROW * (k_lo + i + 1)
        kiota = {}
        for gi, (k_lo, k_hi) in enumerate(GROUPS):
            kio = const_pool.tile([k_hi - k_lo, 1], f32, tag=f"kiota{gi}")
            nc.gpsimd.iota(kio[:], pattern=[[0, 1]], base=ROW * (k_lo + 1),
                           channel_multiplier=ROW,
                           allow_small_or_imprecise_dtypes=True)
            kiota[gi] = kio

        # per-(partition, chunk) maxes, one tile per finale group
        pmax = {}
        for gi, (lo, hi) in enumerate(GROUPS):
            pm_t = small_pool.tile([P, (hi - lo) * C], f32, tag=f"pmax{gi}")
            pmax[gi] = pm_t

        def group_of(k):
            for gi, (lo, hi) in enumerate(GROUPS):
                if lo <= k < hi:
                    return gi
            raise AssertionError(k)

        # -- issue every stream DMA upfront on the sync ring --------------
        cum = [0]
        for g in TILES:
            cum.append(cum[-1] + g)
        tiles_sb = []
        for ti, g in enumerate(TILES):
            t = in_pool.tile([P, g * F], f32, tag="xin")
            nc.sync.dma_start(
                t[:].rearrange("p (g f) -> p g f", f=F),
                x_pkf[:, cum[ti]:cum[ti] + g, :])
            tiles_sb.append(t)

        def reduce_tile(ti):
            k0, g = cum[ti], TILES[ti]
            gi = group_of(k0)
            assert group_of(k0 + g - 1) == gi, "tile spans groups"
            o = k0 - GROUPS[gi][0]
            nc.vector.reduce_max(
                pmax[gi][:, o * C:(o + g) * C],
                tiles_sb[ti][:].rearrange("p (q s) -> p q s", s=S),
                axis=mybir.AxisListType.X)

        st = {}
        gather_engines = [nc.scalar, nc.sync]

        def finale_pre(gi):
            k_lo, k_hi = GROUPS[gi]
            kh = k_hi - k_lo
            pm3 = pmax[gi][:].rearrange("p (k c) -> p k c", c=C)

            # transpose chunk columns -> psumT[k, c*128+p] = chunkmax(p, c)
            psumT = psum_pool.tile([kh, C * P], f32, tag=f"psumT{gi % 2}")
            for c in range(C):
                nc.tensor.matmul(psumT[:, c * P:(c + 1) * P],
                                 pm3[:, :, c], ident[:], is_transpose=True)

            gmax = small_pool.tile([kh, 1], f32, tag=f"gmax{gi}")
            nc.vector.reduce_max(gmax[:], psumT[:], axis=mybir.AxisListType.X)
            cand_p = small_pool.tile([kh, C * P], f32, tag=f"cand_p{gi}")
            nc.vector.scalar_tensor_tensor(
                cand_p[:], in0=psumT[:], scalar=gmax[:],
                in1=riota_pc[0:kh, :], op0=Alu.is_ge, op1=Alu.mult)
            rc = small_pool.tile([kh, 1], f32, tag=f"rc{gi}")  # 256 - chunkrank
            nc.vector.reduce_max(rc[:], cand_p[:], axis=mybir.AxisListType.X)

            # DRAM element offset of winning chunk, per keypoint partition:
            # offs = (256*(k+1) - rc) * 1024 = 262144*(k+1) - 1024*rc
            offs_i = small_pool.tile([kh, 1], i32, tag=f"offs_i{gi}")
            nc.vector.scalar_tensor_tensor(
                offs_i[:], in0=rc[:], scalar=-float(S), in1=kiota[gi][:],
                op0=Alu.mult, op1=Alu.add)

            grow = small_pool.tile([kh, S], f32, tag=f"grow{gi}")
            for k in range(kh):
                eng = gather_engines[k % len(gather_engines)]
                reg = eng.alloc_register()
                eng.load(reg, offs_i[k:k + 1, 0:1])
                off = eng.snap(reg, donate=True)
                eng.dma_start(grow[k:k + 1, :], x_flat[bass.ds(off, S)])

            conf = small_pool.tile([kh, 1], f32, tag=f"conf{gi}")
            nc.scalar.activation(conf[:], gmax[:], Act.Sigmoid)
            st[gi] = (gmax, rc, grow, conf)

        def finale_post(gi):
            k_lo, k_hi = GROUPS[gi]
            kh = k_hi - k_lo
            gmax, rc, grow, conf = st[gi]

            # chunk base = (256 - rc) * 1024, as int
            base_i = small_pool.tile([kh, 1], i32, tag=f"base_i{gi}")
            nc.vector.tensor_scalar(base_i[:], rc[:], -float(S),
                                    float(C * P * S), Alu.mult, Alu.add)
            if USE_MAXIDX:
                # index within the winning chunk via the max_index unit
                gm8 = small_pool.tile([kh, 8], f32, tag=f"gm8{gi}")
                nc.vector.tensor_copy(gm8[:], gmax[:].broadcast_to((kh, 8)))
                idx8 = small_pool.tile([kh, 8], u32, tag=f"idx8{gi}")
                nc.vector.max_index(idx8[:], gm8[:], grow[:])
                flat_i = small_pool.tile([kh, 1], i32, tag=f"flat_i{gi}")
                nc.vector.tensor_tensor(flat_i[:], base_i[:],
                                        idx8[:, 0:1].bitcast(i32),
                                        op=Alu.add)
            else:
                cand_j = small_pool.tile([kh, S], f32, tag=f"cand_j{gi}")
                nc.vector.scalar_tensor_tensor(
                    cand_j[:], in0=grow[:], scalar=gmax[:],
                    in1=riota_j[0:kh, :], op0=Alu.is_ge, op1=Alu.mult)
                rj = small_pool.tile([kh, 1], f32, tag=f"rj{gi}")  # 1024 - j*
                nc.vector.reduce_max(rj[:], cand_j[:],
                                     axis=mybir.AxisListType.X)
                js = small_pool.tile([kh, 1], i32, tag=f"js{gi}")
                nc.vector.tensor_scalar(js[:], rj[:], -1.0, float(S),
                                        Alu.mult, Alu.add)
                flat_i = small_pool.tile([kh, 1], i32, tag=f"flat_i{gi}")
                nc.vector.tensor_tensor(flat_i[:], base_i[:], js[:],
                                        op=Alu.add)

            xx_i = small_pool.tile([kh, 1], i32, tag=f"xx_i{gi}")
            nc.vector.tensor_scalar(xx_i[:], flat_i[:], W - 1, None,
                                    Alu.bitwise_and)
            yy_i = small_pool.tile([kh, 1], i32, tag=f"yy_i{gi}")
            nc.vector.tensor_scalar(yy_i[:], flat_i[:], 9, None,
                                    Alu.logical_shift_right)

            valid = small_pool.tile([kh, 1], f32, tag=f"valid{gi}")
            nc.vector.tensor_scalar(valid[:], conf[:], 0.8, None, Alu.is_gt)

            # out = valid ? (4xx, 4yy, conf) : (-4, -4, -1)
            cand = small_pool.tile([kh, 3], f32, tag=f"cand{gi}")
            nc.vector.tensor_scalar(cand[:, 0:1], xx_i[:], 4.0, None, Alu.mult)
            nc.vector.tensor_scalar(cand[:, 1:2], yy_i[:], 4.0, None, Alu.mult)
            nc.vector.tensor_copy(cand[:, 2:3], conf[:])
            vb3 = small_pool.tile([kh, 3], i32, tag=f"vb3{gi}")
            nc.vector.tensor_scalar(vb3[:], cand[:], 0.0, valid[:],
                                    Alu.mult, Alu.add)
            out_sb = small_pool.tile([kh, 3], f32, tag=f"out_sb{gi}")
            nc.vector.memset(out_sb[:, 0:2], -4.0)
            nc.vector.memset(out_sb[:, 2:3], -1.0)
            nc.vector.copy_predicated(out_sb[:], vb3[:], cand[:])
            nc.scalar.dma_start(out_dram.ap()[k_lo:k_hi, :], out_sb[:])

        # Vector-order emission: each finale op's deps are ready by the
        # time Vector reaches it, so the engine never idles mid-stream.
        reduce_tile(0)
        reduce_tile(1)
        reduce_tile(2)
        finale_pre(0)
        reduce_tile(3)
        finale_post(0)
        reduce_tile(4)
        finale_pre(1)
        reduce_tile(5)
        finale_post(1)
        finale_pre(2)
        reduce_tile(6)
        finale_pre(3)
        finale_post(3)
        finale_post(2)

    nc.compile()
    return nc


def _get_nc():
    global _NC_CACHE
    if _NC_CACHE is None:
        _NC_CACHE = _build()
    return _NC_CACHE


def _shard(x: np.ndarray) -> list[dict[str, np.ndarray]]:
    xf = np.ascontiguousarray(np.asarray(x, dtype=np.float32).reshape(NK, ROW))
    shards = []
    for c in range(N_CORES):
        lo = c * K
        s = xf[lo:min(lo + K, NK)]
        if s.shape[0] < K:
            s = np.concatenate(
                [s, np.zeros((K - s.shape[0], ROW), np.float32)], axis=0)
        shards.append({"x": np.ascontiguousarray(s)})
    return shards


def _run(x, trace=False, **kw):
    nc = _get_nc()
    res = run_bass_kernel_spmd(nc, _shard(x), core_ids=list(range(N_CORES)),
                               trace=trace, **kw)
    out = np.concatenate([r["out"] for r in res.results], axis=0)[:NK]
    return out.astype(np.float32), res


def kernel(x: np.ndarray) -> np.ndarray:
    out, _ = _run(x, trace=False)
    return out


# revision 17
# speedup vs baseline: 1.2113x; 1.0267x over previous
"""Trainium2 Bass kernel for nn_DecodeSBP (keypoint heatmap decode).

Contract: kernel(x=[1,133,512,512] f32) -> [133,3] f32
  joints[k] = (4*xx, 4*yy, conf) if conf > 0.8 else (-4, -4, -1)
  where flat = argmax(sigmoid(x[0,k])), conf = sigmoid(max), yy = flat//512,
  xx = flat%512. sigmoid is monotonic so the argmax runs on raw logits.

Sharding: keypoint dim across 8 cores (17/core, core 7 zero-padded).

Per-core program (one full-data pass, hierarchical argmax), v3:
  All 7 stream-tile DMAs are issued upfront on the sync ring (nothing can
  delay them); the DVE chunk-max reduces and the per-group finale chains
  are interleaved in an order chosen so Vector never waits on a gather
  or transpose. Finale groups (5,6,5,1 keypoints): PE-transpose ->
  global max -> rank-trick chunk argmax -> register-offset gather of the
  winning 4KB chunk (scalar+sync sequencers only; gpsimd SWDGE completion
  sems measured ~6us late) -> in-chunk argmax (max_index) -> integer
  decode. The last group is the single last-streamed keypoint, so the
  post-stream tail is one short chain.
"""

import sys
from contextlib import ExitStack

for _p in ("/opt/trn_rl_repo", "/opt/pypackages"):
    if _p not in sys.path:
        sys.path.append(_p)

import numpy as np

import concourse.bacc as bacc
import concourse.bass as bass
import concourse.tile as tile
from concourse import mybir
from concourse.bass_utils import run_bass_kernel_spmd
from concourse.masks import make_identity

K = 17          # keypoints per core
NK = 133        # total keypoints
ROW = 262144    # 512*512
P = 128         # SBUF partitions
F = ROW // P    # 2048 free elems per partition
C = 2           # chunks per partition row
S = F // C      # 1024 elems per chunk
W = 512
N_CORES = 8
TILES = (2, 3, 3, 3, 3, 2, 1)           # keypoints per stream tile (sum 17)
GROUPS = ((0, 5), (5, 11), (11, 16), (16, 17))  # finale groups [k_lo, k_hi)
MAXG = max(hi - lo for lo, hi in GROUPS)

f32 = mybir.dt.float32
i32 = mybir.dt.int32
u32 = mybir.dt.uint32
Alu = mybir.AluOpType
Act = mybir.ActivationFunctionType

USE_MAXIDX = True

_NC_CACHE = None


def _build():
    nc = bacc.Bacc("TRN2", target_bir_lowering=False, debug=False)
    x_dram = nc.dram_tensor("x", [K, ROW], f32, kind="ExternalInput")
    out_dram = nc.dram_tensor("out", [K, 3], f32, kind="ExternalOutput")

    x_pkf = x_dram.ap().rearrange("k (p f) -> p k f", f=F)      # [128, K, 2048]
    x_flat = x_dram.ap().rearrange("k f -> (k f)")

    with tile.TileContext(nc) as tc, ExitStack() as ctx:
        const_pool = ctx.enter_context(tc.tile_pool(name="const", bufs=1))
        in_pool = ctx.enter_context(
            tc.tile_pool(name="in", bufs=len(TILES) - 1))
        small_pool = ctx.enter_context(tc.tile_pool(name="small", bufs=1))
        psum_pool = ctx.enter_context(
            tc.tile_pool(name="psum", bufs=2, space="PSUM"))

        ident = const_pool.tile([P, P], f32)
        make_identity(nc, ident[:])
        # riota_pc[k, c*128+p] = 256 - (2p + c): rank of chunk (p,c) in flat
        # order, reversed so reduce_max picks the first occurrence.
        riota_pc = const_pool.tile([MAXG, C * P], f32)
        nc.gpsimd.iota(riota_pc[:].rearrange("k (c p) -> k c p", p=P),
                       pattern=[[-1, C], [-C, P]], base=C * P,
                       channel_multiplier=0,
                       allow_small_or_imprecise_dtypes=True)
        # riota_j[k, j] = 1024 - j  (rank-trick fallback when not USE_MAXIDX)
        riota_j = const_pool.tile([MAXG, S], f32)
        nc.gpsimd.iota(riota_j[:], pattern=[[-1, S]], base=S,
                       channel_multiplier=0,
                       allow_small_or_imprecise_dtypes=True)
        # kiota[g][i, 0] = ROW * (k_lo + i + 1)
        kiota = {}
        for gi, (k_lo, k_hi) in enumerate(GROUPS):
            kio = const_pool.tile([k_hi - k_lo, 1], f32, tag=f"kiota{gi}")
            nc.gpsimd.iota(kio[:], pattern=[[0, 1]], base=ROW * (k_lo + 1),
                           channel_multiplier=ROW,
                           allow_small_or_imprecise_dtypes=True)
            kiota[gi] = kio

        # per-(partition, chunk) maxes, one tile per finale group
        pmax = {}
        for gi, (lo, hi) in enumerate(GROUPS):
            pm_t = small_pool.tile([P, (hi - lo) * C], f32, tag=f"pmax{gi}")
            pmax[gi] = pm_t

        def group_of(k):
            for gi, (lo, hi) in enumerate(GROUPS):
                if lo <= k < hi:
                    return gi
            raise AssertionError(k)

        # -- issue every stream DMA upfront on the sync ring --------------
        cum = [0]
        for g in TILES:
            cum.append(cum[-1] + g)
        tiles_sb = []
        for ti, g in enumerate(TILES):
            t = in_pool.tile([P, g * F], f32, tag="xin")
            nc.sync.dma_start(
                t[:].rearrange("p (g f) -> p g f", f=F),
                x_pkf[:, cum[ti]:cum[ti] + g, :])
            tiles_sb.append(t)

        def reduce_tile(ti):
            k0, g = cum[ti], TILES[ti]
            gi = group_of(k0)
            assert group_of(k0 + g - 1) == gi, "tile spans groups"
            o = k0 - GROUPS[gi][0]
            nc.vector.reduce_max(
                pmax[gi][:, o * C:(o + g) * C],
                tiles_sb[ti][:].rearrange("p (q s) -> p q s", s=S),
                axis=mybir.AxisListType.X)

        st = {}
        gather_engines = [nc.scalar, nc.sync]

        def finale_pre(gi):
            k_lo, k_hi = GROUPS[gi]
            kh = k_hi - k_lo
            pm3 = pmax[gi][:].rearrange("p (k c) -> p k c", c=C)

            # transpose chunk columns -> psumT[k, c*128+p] = chunkmax(p, c)
            psumT = psum_pool.tile([kh, C * P], f32, tag=f"psumT{gi % 2}")
            for c in range(C):
                nc.tensor.matmul(psumT[:, c * P:(c + 1) * P],
                                 pm3[:, :, c], ident[:], is_transpose=True)

            gmax = small_pool.tile([kh, 1], f32, tag=f"gmax{gi}")
            nc.vector.reduce_max(gmax[:], psumT[:], axis=mybir.AxisListType.X)
            cand_p = small_pool.tile([kh, C * P], f32, tag=f"cand_p{gi}")
            nc.vector.scalar_tensor_tensor(
                cand_p[:], in0=psumT[:], scalar=gmax[:],
                in1=riota_pc[0:kh, :], op0=Alu.is_ge, op1=Alu.mult)
            rc = small_pool.tile([kh, 1], f32, tag=f"rc{gi}")  # 256 - chunkrank
            nc.vector.reduce_max(rc[:], cand_p[:], axis=mybir.AxisListType.X)

            # DRAM element offset of winning chunk, per keypoint partition:
            # offs = (256*(k+1) - rc) * 1024 = 262144*(k+1) - 1024*rc
            offs_i = small_pool.tile([kh, 1], i32, tag=f"offs_i{gi}")
            nc.vector.scalar_tensor_tensor(
                offs_i[:], in0=rc[:], scalar=-float(S), in1=kiota[gi][:],
                op0=Alu.mult, op1=Alu.add)

            grow = small_pool.tile([kh, S], f32, tag=f"grow{gi}")
            for k in range(kh):
                eng = gather_engines[k % len(gather_engines)]
                reg = eng.alloc_register()
                eng.load(reg, offs_i[k:k + 1, 0:1])
                off = eng.snap(reg, donate=True)
                eng.dma_start(grow[k:k + 1, :], x_flat[bass.ds(off, S)])

            conf = small_pool.tile([kh, 1], f32, tag=f"conf{gi}")
            nc.scalar.activation(conf[:], gmax[:], Act.Sigmoid)
            st[gi] = (gmax, rc, grow, conf)

        def finale_post(gi, out_eng):
            k_lo, k_hi = GROUPS[gi]
            kh = k_hi - k_lo
            gmax, rc, grow, conf = st[gi]

            # chunk base = (256 - rc) * 1024, as int
            base_i = small_pool.tile([kh, 1], i32, tag=f"base_i{gi}")
            nc.vector.tensor_scalar(base_i[:], rc[:], -float(S),
                                    float(C * P * S), Alu.mult, Alu.add)
            if USE_MAXIDX:
                # index within the winning chunk via the max_index unit
                gm8 = small_pool.tile([kh, 8], f32, tag=f"gm8{gi}")
                nc.vector.tensor_copy(gm8[:], gmax[:].broadcast_to((kh, 8)))
                idx8 = small_pool.tile([kh, 8], u32, tag=f"idx8{gi}")
                nc.vector.max_index(idx8[:], gm8[:], grow[:])
                flat_i = small_pool.tile([kh, 1], i32, tag=f"flat_i{gi}")
                nc.vector.tensor_tensor(flat_i[:], base_i[:],
                                        idx8[:, 0:1].bitcast(i32),
                                        op=Alu.add)
            else:
                cand_j = small_pool.tile([kh, S], f32, tag=f"cand_j{gi}")
                nc.vector.scalar_tensor_tensor(
                    cand_j[:], in0=grow[:], scalar=gmax[:],
                    in1=riota_j[0:kh, :], op0=Alu.is_ge, op1=Alu.mult)
                rj = small_pool.tile([kh, 1], f32, tag=f"rj{gi}")  # 1024 - j*
                nc.vector.reduce_max(rj[:], cand_j[:],
                                     axis=mybir.AxisListType.X)
                js = small_pool.tile([kh, 1], i32, tag=f"js{gi}")
                nc.vector.tensor_scalar(js[:], rj[:], -1.0, float(S),
                                        Alu.mult, Alu.add)
                flat_i = small_pool.tile([kh, 1], i32, tag=f"flat_i{gi}")
                nc.vector.tensor_tensor(flat_i[:], base_i[:], js[:],
                                        op=Alu.add)

            xx_i = small_pool.tile([kh, 1], i32, tag=f"xx_i{gi}")
            nc.vector.tensor_scalar(xx_i[:], flat_i[:], W - 1, None,
                                    Alu.bitwise_and)
            yy_i = small_pool.tile([kh, 1], i32, tag=f"yy_i{gi}")
            nc.vector.tensor_scalar(yy_i[:], flat_i[:], 9, None,
                                    Alu.logical_shift_right)

            valid = small_pool.tile([kh, 1], f32, tag=f"valid{gi}")
            nc.vector.tensor_scalar(valid[:], conf[:], 0.8, None, Alu.is_gt)

            # out = valid ? (4xx, 4yy, conf) : (-4, -4, -1)
            cand = small_pool.tile([kh, 3], f32, tag=f"cand{gi}")
            nc.vector.tensor_scalar(cand[:, 0:1], xx_i[:], 4.0, None, Alu.mult)
            nc.vector.tensor_scalar(cand[:, 1:2], yy_i[:], 4.0, None, Alu.mult)
            nc.vector.tensor_copy(cand[:, 2:3], conf[:])
            vb3 = small_pool.tile([kh, 3], i32, tag=f"vb3{gi}")
            nc.vector.tensor_scalar(vb3[:], cand[:], 0.0, valid[:],
                                    Alu.mult, Alu.add)
            out_sb = small_pool.tile([kh, 3], f32, tag=f"out_sb{gi}")
            nc.vector.memset(out_sb[:, 0:2], -4.0)
            nc.vector.memset(out_sb[:, 2:3], -1.0)
            nc.vector.copy_predicated(out_sb[:], vb3[:], cand[:])
            # early groups write out via gpsimd SWDGE so the scalar ring's
            # in-order program can't stall later groups' gathers behind them
            out_eng.dma_start(out_dram.ap()[k_lo:k_hi, :], out_sb[:])

        # Vector-order emission: each finale op's deps are ready by the
        # time Vector reaches it, so the engine never idles mid-stream.
        reduce_tile(0)
        reduce_tile(1)
        reduce_tile(2)
        finale_pre(0)
        reduce_tile(3)
        finale_post(0, nc.gpsimd)
        reduce_tile(4)
        finale_pre(1)
        reduce_tile(5)
        finale_post(1, nc.gpsimd)
        finale_pre(2)
        reduce_tile(6)
        finale_pre(3)
        finale_post(3, nc.scalar)
        finale_post(2, nc.scalar)

    nc.compile()
    return nc


def _get_nc():
    global _NC_CACHE
    if _NC_CACHE is None:
        _NC_CACHE = _build()
    return _NC_CACHE


def _shard(x: np.ndarray) -> list[dict[str, np.ndarray]]:
    xf = np.ascontiguousarray(np.asarray(x, dtype=np.float32).reshape(NK, ROW))
    shards = []
    for c in range(N_CORES):
        lo = c * K
        s = xf[lo:min(lo + K, NK)]
        if s.shape[0] < K:
            s = np.concatenate(
                [s, np.zeros((K - s.shape[0], ROW), np.float32)], axis=0)
        shards.append({"x": np.ascontiguousarray(s)})
    return shards


def _run(x, trace=False, **kw):
    nc = _get_nc()
    res = run_bass_kernel_spmd(nc, _shard(x), core_ids=list(range(N_CORES)),
                               trace=trace, **kw)
    out = np.concatenate([r["out"] for r in res.results], axis=0)[:NK]
    return out.astype(np.float32), res


def kernel(x: np.ndarray) -> np.ndarray:
    out, _ = _run(x, trace=False)
    return out


# revision 19
# speedup vs baseline: 1.2389x; 1.0228x over previous
"""Trainium2 Bass kernel for nn_DecodeSBP (keypoint heatmap decode).

Contract: kernel(x=[1,133,512,512] f32) -> [133,3] f32
  joints[k] = (4*xx, 4*yy, conf) if conf > 0.8 else (-4, -4, -1)
  where flat = argmax(sigmoid(x[0,k])), conf = sigmoid(max), yy = flat//512,
  xx = flat%512. sigmoid is monotonic so the argmax runs on raw logits.

Sharding: keypoint dim across 8 cores (17/core, core 7 zero-padded).

Per-core program (one full-data pass, hierarchical argmax), v3:
  All 7 stream-tile DMAs are issued upfront on the sync ring (nothing can
  delay them); the DVE chunk-max reduces and the per-group finale chains
  are interleaved in an order chosen so Vector never waits on a gather
  or transpose. Finale groups (5,6,5,1 keypoints): PE-transpose ->
  global max -> rank-trick chunk argmax -> register-offset gather of the
  winning 4KB chunk (scalar+sync sequencers only; gpsimd SWDGE completion
  sems measured ~6us late) -> in-chunk argmax (max_index) -> integer
  decode. The last group is the single last-streamed keypoint, so the
  post-stream tail is one short chain.
"""

import sys
from contextlib import ExitStack

for _p in ("/opt/trn_rl_repo", "/opt/pypackages"):
    if _p not in sys.path:
        sys.path.append(_p)

import numpy as np

import concourse.bacc as bacc
import concourse.bass as bass
import concourse.tile as tile
from concourse import mybir
from concourse.bass_utils import run_bass_kernel_spmd
from concourse.masks import make_identity

K = 17          # keypoints per core
NK = 133        # total keypoints
ROW = 262144    # 512*512
P = 128         # SBUF partitions
F = ROW // P    # 2048 free elems per partition
C = 2           # chunks per partition row
S = F // C      # 1024 elems per chunk
W = 512
N_CORES = 8
TILES = (2, 3, 3, 3, 3, 2, 1)           # keypoints per stream tile (sum 17)
GROUPS = ((0, 5), (5, 11), (11, 14), (14, 17))  # finale groups [k_lo, k_hi)
MAXG = max(hi - lo for lo, hi in GROUPS)

f32 = mybir.dt.float32
i32 = mybir.dt.int32
u32 = mybir.dt.uint32
Alu = mybir.AluOpType
Act = mybir.ActivationFunctionType

USE_MAXIDX = True

_NC_CACHE = None


def _build():
    nc = bacc.Bacc("TRN2", target_bir_lowering=False, debug=False)
    x_dram = nc.dram_tensor("x", [K, ROW], f32, kind="ExternalInput")
    out_dram = nc.dram_tensor("out", [K, 3], f32, kind="ExternalOutput")

    x_pkf = x_dram.ap().rearrange("k (p f) -> p k f", f=F)      # [128, K, 2048]
    x_flat = x_dram.ap().rearrange("k f -> (k f)")

    with tile.TileContext(nc) as tc, ExitStack() as ctx:
        const_pool = ctx.enter_context(tc.tile_pool(name="const", bufs=1))
        in_pool = ctx.enter_context(
            tc.tile_pool(name="in", bufs=len(TILES) - 1))
        small_pool = ctx.enter_context(tc.tile_pool(name="small", bufs=1))
        psum_pool = ctx.enter_context(
            tc.tile_pool(name="psum", bufs=2, space="PSUM"))

        ident = const_pool.tile([P, P], f32)
        make_identity(nc, ident[:])
        # riota_pc[k, c*128+p] = 256 - (2p + c): rank of chunk (p,c) in flat
        # order, reversed so reduce_max picks the first occurrence.
        riota_pc = const_pool.tile([MAXG, C * P], f32)
        nc.gpsimd.iota(riota_pc[:].rearrange("k (c p) -> k c p", p=P),
                       pattern=[[-1, C], [-C, P]], base=C * P,
                       channel_multiplier=0,
                       allow_small_or_imprecise_dtypes=True)
        # riota_j[k, j] = 1024 - j  (rank-trick fallback when not USE_MAXIDX)
        riota_j = const_pool.tile([MAXG, S], f32)
        nc.gpsimd.iota(riota_j[:], pattern=[[-1, S]], base=S,
                       channel_multiplier=0,
                       allow_small_or_imprecise_dtypes=True)
        # kiota[g][i, 0] = ROW * (k_lo + i + 1)
        kiota = {}
        for gi, (k_lo, k_hi) in enumerate(GROUPS):
            kio = const_pool.tile([k_hi - k_lo, 1], f32, tag=f"kiota{gi}")
            nc.gpsimd.iota(kio[:], pattern=[[0, 1]], base=ROW * (k_lo + 1),
                           channel_multiplier=ROW,
                           allow_small_or_imprecise_dtypes=True)
            kiota[gi] = kio

        # per-(partition, chunk) maxes, one tile per finale group
        pmax = {}
        for gi, (lo, hi) in enumerate(GROUPS):
            pm_t = small_pool.tile([P, (hi - lo) * C], f32, tag=f"pmax{gi}")
            pmax[gi] = pm_t

        def group_of(k):
            for gi, (lo, hi) in enumerate(GROUPS):
                if lo <= k < hi:
                    return gi
            raise AssertionError(k)

        # -- issue every stream DMA upfront on the sync ring --------------
        cum = [0]
        for g in TILES:
            cum.append(cum[-1] + g)
        tiles_sb = []
        for ti, g in enumerate(TILES):
            t = in_pool.tile([P, g * F], f32, tag="xin")
            nc.sync.dma_start(
                t[:].rearrange("p (g f) -> p g f", f=F),
                x_pkf[:, cum[ti]:cum[ti] + g, :])
            tiles_sb.append(t)

        def reduce_tile(ti):
            k0, g = cum[ti], TILES[ti]
            gi = group_of(k0)
            assert group_of(k0 + g - 1) == gi, "tile spans groups"
            o = k0 - GROUPS[gi][0]
            nc.vector.reduce_max(
                pmax[gi][:, o * C:(o + g) * C],
                tiles_sb[ti][:].rearrange("p (q s) -> p q s", s=S),
                axis=mybir.AxisListType.X)

        st = {}
        gather_engines = [nc.scalar, nc.sync]

        def finale_pre(gi):
            k_lo, k_hi = GROUPS[gi]
            kh = k_hi - k_lo
            pm3 = pmax[gi][:].rearrange("p (k c) -> p k c", c=C)

            # transpose chunk columns -> psumT[k, c*128+p] = chunkmax(p, c)
            psumT = psum_pool.tile([kh, C * P], f32, tag=f"psumT{gi % 2}")
            for c in range(C):
                nc.tensor.matmul(psumT[:, c * P:(c + 1) * P],
                                 pm3[:, :, c], ident[:], is_transpose=True)

            gmax = small_pool.tile([kh, 1], f32, tag=f"gmax{gi}")
            nc.vector.reduce_max(gmax[:], psumT[:], axis=mybir.AxisListType.X)
            cand_p = small_pool.tile([kh, C * P], f32, tag=f"cand_p{gi}")
            nc.vector.scalar_tensor_tensor(
                cand_p[:], in0=psumT[:], scalar=gmax[:],
                in1=riota_pc[0:kh, :], op0=Alu.is_ge, op1=Alu.mult)
            rc = small_pool.tile([kh, 1], f32, tag=f"rc{gi}")  # 256 - chunkrank
            nc.vector.reduce_max(rc[:], cand_p[:], axis=mybir.AxisListType.X)

            # DRAM element offset of winning chunk, per keypoint partition:
            # offs = (256*(k+1) - rc) * 1024 = 262144*(k+1) - 1024*rc
            offs_i = small_pool.tile([kh, 1], i32, tag=f"offs_i{gi}")
            nc.vector.scalar_tensor_tensor(
                offs_i[:], in0=rc[:], scalar=-float(S), in1=kiota[gi][:],
                op0=Alu.mult, op1=Alu.add)

            grow = small_pool.tile([kh, S], f32, tag=f"grow{gi}")
            for k in range(kh):
                eng = gather_engines[k % len(gather_engines)]
                reg = eng.alloc_register()
                eng.load(reg, offs_i[k:k + 1, 0:1])
                off = eng.snap(reg, donate=True)
                eng.dma_start(grow[k:k + 1, :], x_flat[bass.ds(off, S)])

            conf = small_pool.tile([kh, 1], f32, tag=f"conf{gi}")
            nc.scalar.activation(conf[:], gmax[:], Act.Sigmoid)
            st[gi] = (gmax, rc, grow, conf)

        def finale_post(gi, out_eng):
            k_lo, k_hi = GROUPS[gi]
            kh = k_hi - k_lo
            gmax, rc, grow, conf = st[gi]

            # chunk base = (256 - rc) * 1024, as int
            base_i = small_pool.tile([kh, 1], i32, tag=f"base_i{gi}")
            nc.vector.tensor_scalar(base_i[:], rc[:], -float(S),
                                    float(C * P * S), Alu.mult, Alu.add)
            if USE_MAXIDX:
                # index within the winning chunk via the max_index unit
                gm8 = small_pool.tile([kh, 8], f32, tag=f"gm8{gi}")
                nc.vector.tensor_copy(gm8[:], gmax[:].broadcast_to((kh, 8)))
                idx8 = small_pool.tile([kh, 8], u32, tag=f"idx8{gi}")
                nc.vector.max_index(idx8[:], gm8[:], grow[:])
                flat_i = small_pool.tile([kh, 1], i32, tag=f"flat_i{gi}")
                nc.vector.tensor_tensor(flat_i[:], base_i[:],
                                        idx8[:, 0:1].bitcast(i32),
                                        op=Alu.add)
            else:
                cand_j = small_pool.tile([kh, S], f32, tag=f"cand_j{gi}")
                nc.vector.scalar_tensor_tensor(
                    cand_j[:], in0=grow[:], scalar=gmax[:],
                    in1=riota_j[0:kh, :], op0=Alu.is_ge, op1=Alu.mult)
                rj = small_pool.tile([kh, 1], f32, tag=f"rj{gi}")  # 1024 - j*
                nc.vector.reduce_max(rj[:], cand_j[:],
                                     axis=mybir.AxisListType.X)
                js = small_pool.tile([kh, 1], i32, tag=f"js{gi}")
                nc.vector.tensor_scalar(js[:], rj[:], -1.0, float(S),
                                        Alu.mult, Alu.add)
                flat_i = small_pool.tile([kh, 1], i32, tag=f"flat_i{gi}")
                nc.vector.tensor_tensor(flat_i[:], base_i[:], js[:],
                                        op=Alu.add)

            xx_i = small_pool.tile([kh, 1], i32, tag=f"xx_i{gi}")
            nc.vector.tensor_scalar(xx_i[:], flat_i[:], W - 1, None,
                                    Alu.bitwise_and)
            yy_i = small_pool.tile([kh, 1], i32, tag=f"yy_i{gi}")
            nc.vector.tensor_scalar(yy_i[:], flat_i[:], 9, None,
                                    Alu.logical_shift_right)

            valid = small_pool.tile([kh, 1], f32, tag=f"valid{gi}")
            nc.vector.tensor_scalar(valid[:], conf[:], 0.8, None, Alu.is_gt)

            # out = valid ? (4xx, 4yy, conf) : (-4, -4, -1)
            cand = small_pool.tile([kh, 3], f32, tag=f"cand{gi}")
            nc.vector.tensor_scalar(cand[:, 0:1], xx_i[:], 4.0, None, Alu.mult)
            nc.vector.tensor_scalar(cand[:, 1:2], yy_i[:], 4.0, None, Alu.mult)
            nc.vector.tensor_copy(cand[:, 2:3], conf[:])
            vb3 = small_pool.tile([kh, 3], i32, tag=f"vb3{gi}")
            nc.vector.tensor_scalar(vb3[:], cand[:], 0.0, valid[:],
                                    Alu.mult, Alu.add)
            out_sb = small_pool.tile([kh, 3], f32, tag=f"out_sb{gi}")
            nc.vector.memset(out_sb[:, 0:2], -4.0)
            nc.vector.memset(out_sb[:, 2:3], -1.0)
            nc.vector.copy_predicated(out_sb[:], vb3[:], cand[:])
            # early groups write out via gpsimd SWDGE so the scalar ring's
            # in-order program can't stall later groups' gathers behind them
            out_eng.dma_start(out_dram.ap()[k_lo:k_hi, :], out_sb[:])

        # Vector-order emission: each finale op's deps are ready by the
        # time Vector reaches it, so the engine never idles mid-stream.
        reduce_tile(0)
        reduce_tile(1)
        reduce_tile(2)
        finale_pre(0)
        reduce_tile(3)
        finale_post(0, nc.gpsimd)
        reduce_tile(4)
        finale_pre(1)
        finale_pre(2)
        reduce_tile(5)
        finale_post(1, nc.gpsimd)
        finale_post(2, nc.scalar)
        reduce_tile(6)
        finale_pre(3)
        finale_post(3, nc.scalar)

    nc.compile()
    return nc


def _get_nc():
    global _NC_CACHE
    if _NC_CACHE is None:
        _NC_CACHE = _build()
    return _NC_CACHE


def _shard(x: np.ndarray) -> list[dict[str, np.ndarray]]:
    xf = np.ascontiguousarray(np.asarray(x, dtype=np.float32).reshape(NK, ROW))
    shards = []
    for c in range(N_CORES):
        lo = c * K
        s = xf[lo:min(lo + K, NK)]
        if s.shape[0] < K:
            s = np.concatenate(
                [s, np.zeros((K - s.shape[0], ROW), np.float32)], axis=0)
        shards.append({"x": np.ascontiguousarray(s)})
    return shards


def _run(x, trace=False, **kw):
    nc = _get_nc()
    res = run_bass_kernel_spmd(nc, _shard(x), core_ids=list(range(N_CORES)),
                               trace=trace, **kw)
    out = np.concatenate([r["out"] for r in res.results], axis=0)[:NK]
    return out.astype(np.float32), res


def kernel(x: np.ndarray) -> np.ndarray:
    out, _ = _run(x, trace=False)
    return out


# revision 20
# speedup vs baseline: 1.2553x; 1.0133x over previous
"""Trainium2 Bass kernel for nn_DecodeSBP (keypoint heatmap decode).

Contract: kernel(x=[1,133,512,512] f32) -> [133,3] f32
  joints[k] = (4*xx, 4*yy, conf) if conf > 0.8 else (-4, -4, -1)
  where flat = argmax(sigmoid(x[0,k])), conf = sigmoid(max), yy = flat//512,
  xx = flat%512. sigmoid is monotonic so the argmax runs on raw logits.

Sharding: keypoint dim across 8 cores (17/core, core 7 zero-padded).

Per-core program (one full-data pass, hierarchical argmax), v3:
  All 7 stream-tile DMAs are issued upfront on the sync ring (nothing can
  delay them); the DVE chunk-max reduces and the per-group finale chains
  are interleaved in an order chosen so Vector never waits on a gather
  or transpose. Finale groups (5,6,5,1 keypoints): PE-transpose ->
  global max -> rank-trick chunk argmax -> register-offset gather of the
  winning 4KB chunk (scalar+sync sequencers only; gpsimd SWDGE completion
  sems measured ~6us late) -> in-chunk argmax (max_index) -> integer
  decode. The last group is the single last-streamed keypoint, so the
  post-stream tail is one short chain.
"""

import sys
from contextlib import ExitStack

for _p in ("/opt/trn_rl_repo", "/opt/pypackages"):
    if _p not in sys.path:
        sys.path.append(_p)

import numpy as np

import concourse.bacc as bacc
import concourse.bass as bass
import concourse.tile as tile
from concourse import mybir
from concourse.bass_utils import run_bass_kernel_spmd
from concourse.masks import make_identity

K = 17          # keypoints per core
NK = 133        # total keypoints
ROW = 262144    # 512*512
P = 128         # SBUF partitions
F = ROW // P    # 2048 free elems per partition
C = 2           # chunks per partition row
S = F // C      # 1024 elems per chunk
W = 512
N_CORES = 8
TILES = (2, 3, 3, 3, 3, 2, 1)           # keypoints per stream tile (sum 17)
GROUPS = ((0, 5), (5, 11), (11, 14), (14, 17))  # finale groups [k_lo, k_hi)
MAXG = max(hi - lo for lo, hi in GROUPS)

f32 = mybir.dt.float32
i32 = mybir.dt.int32
u32 = mybir.dt.uint32
Alu = mybir.AluOpType
Act = mybir.ActivationFunctionType

USE_MAXIDX = True

_NC_CACHE = None


def _build():
    nc = bacc.Bacc("TRN2", target_bir_lowering=False, debug=False)
    x_dram = nc.dram_tensor("x", [K, ROW], f32, kind="ExternalInput")
    out_dram = nc.dram_tensor("out", [K, 3], f32, kind="ExternalOutput")

    x_pkf = x_dram.ap().rearrange("k (p f) -> p k f", f=F)      # [128, K, 2048]
    x_flat = x_dram.ap().rearrange("k f -> (k f)")

    with tile.TileContext(nc) as tc, ExitStack() as ctx:
        const_pool = ctx.enter_context(tc.tile_pool(name="const", bufs=1))
        in_pool = ctx.enter_context(
            tc.tile_pool(name="in", bufs=len(TILES) - 1))
        small_pool = ctx.enter_context(tc.tile_pool(name="small", bufs=1))
        psum_pool = ctx.enter_context(
            tc.tile_pool(name="psum", bufs=2, space="PSUM"))

        ident = const_pool.tile([P, P], f32)
        make_identity(nc, ident[:])
        # riota_pc[k, c*128+p] = 256 - (2p + c): rank of chunk (p,c) in flat
        # order, reversed so reduce_max picks the first occurrence.
        riota_pc = const_pool.tile([MAXG, C * P], f32)
        nc.gpsimd.iota(riota_pc[:].rearrange("k (c p) -> k c p", p=P),
                       pattern=[[-1, C], [-C, P]], base=C * P,
                       channel_multiplier=0,
                       allow_small_or_imprecise_dtypes=True)
        # riota_j[k, j] = 1024 - j  (rank-trick fallback when not USE_MAXIDX)
        riota_j = const_pool.tile([MAXG, S], f32)
        nc.gpsimd.iota(riota_j[:], pattern=[[-1, S]], base=S,
                       channel_multiplier=0,
                       allow_small_or_imprecise_dtypes=True)
        # kiota[g][i, 0] = ROW * (k_lo + i + 1)
        kiota = {}
        for gi, (k_lo, k_hi) in enumerate(GROUPS):
            kio = const_pool.tile([k_hi - k_lo, 1], f32, tag=f"kiota{gi}")
            nc.gpsimd.iota(kio[:], pattern=[[0, 1]], base=ROW * (k_lo + 1),
                           channel_multiplier=ROW,
                           allow_small_or_imprecise_dtypes=True)
            kiota[gi] = kio

        # per-(partition, chunk) maxes, one tile per finale group
        pmax = {}
        for gi, (lo, hi) in enumerate(GROUPS):
            pm_t = small_pool.tile([P, (hi - lo) * C], f32, tag=f"pmax{gi}")
            pmax[gi] = pm_t

        def group_of(k):
            for gi, (lo, hi) in enumerate(GROUPS):
                if lo <= k < hi:
                    return gi
            raise AssertionError(k)

        # -- issue every stream DMA upfront on the sync ring --------------
        cum = [0]
        for g in TILES:
            cum.append(cum[-1] + g)
        tiles_sb = []
        for ti, g in enumerate(TILES):
            t = in_pool.tile([P, g * F], f32, tag="xin")
            nc.sync.dma_start(
                t[:].rearrange("p (g f) -> p g f", f=F),
                x_pkf[:, cum[ti]:cum[ti] + g, :])
            tiles_sb.append(t)

        def reduce_tile(ti):
            k0, g = cum[ti], TILES[ti]
            gi = group_of(k0)
            assert group_of(k0 + g - 1) == gi, "tile spans groups"
            o = k0 - GROUPS[gi][0]
            nc.vector.reduce_max(
                pmax[gi][:, o * C:(o + g) * C],
                tiles_sb[ti][:].rearrange("p (q s) -> p q s", s=S),
                axis=mybir.AxisListType.X)

        st = {}
        gather_engines = [nc.sync, nc.scalar]

        def finale_pre(gi):
            k_lo, k_hi = GROUPS[gi]
            kh = k_hi - k_lo
            pm3 = pmax[gi][:].rearrange("p (k c) -> p k c", c=C)

            # transpose chunk columns -> psumT[k, c*128+p] = chunkmax(p, c)
            psumT = psum_pool.tile([kh, C * P], f32, tag=f"psumT{gi % 2}")
            for c in range(C):
                nc.tensor.matmul(psumT[:, c * P:(c + 1) * P],
                                 pm3[:, :, c], ident[:], is_transpose=True)

            gmax = small_pool.tile([kh, 1], f32, tag=f"gmax{gi}")
            nc.vector.reduce_max(gmax[:], psumT[:], axis=mybir.AxisListType.X)
            cand_p = small_pool.tile([kh, C * P], f32, tag=f"cand_p{gi}")
            nc.vector.scalar_tensor_tensor(
                cand_p[:], in0=psumT[:], scalar=gmax[:],
                in1=riota_pc[0:kh, :], op0=Alu.is_ge, op1=Alu.mult)
            rc = small_pool.tile([kh, 1], f32, tag=f"rc{gi}")  # 256 - chunkrank
            nc.vector.reduce_max(rc[:], cand_p[:], axis=mybir.AxisListType.X)

            # DRAM element offset of winning chunk, per keypoint partition:
            # offs = (256*(k+1) - rc) * 1024 = 262144*(k+1) - 1024*rc
            offs_i = small_pool.tile([kh, 1], i32, tag=f"offs_i{gi}")
            nc.vector.scalar_tensor_tensor(
                offs_i[:], in0=rc[:], scalar=-float(S), in1=kiota[gi][:],
                op0=Alu.mult, op1=Alu.add)

            grow = small_pool.tile([kh, S], f32, tag=f"grow{gi}")
            for k in range(kh):
                eng = gather_engines[k % len(gather_engines)]
                reg = eng.alloc_register()
                eng.load(reg, offs_i[k:k + 1, 0:1])
                off = eng.snap(reg, donate=True)
                eng.dma_start(grow[k:k + 1, :], x_flat[bass.ds(off, S)])

            conf = small_pool.tile([kh, 1], f32, tag=f"conf{gi}")
            nc.scalar.activation(conf[:], gmax[:], Act.Sigmoid)
            st[gi] = (gmax, rc, grow, conf)

        def finale_post(gi, out_eng):
            k_lo, k_hi = GROUPS[gi]
            kh = k_hi - k_lo
            gmax, rc, grow, conf = st[gi]

            # chunk base = (256 - rc) * 1024, as int
            base_i = small_pool.tile([kh, 1], i32, tag=f"base_i{gi}")
            nc.vector.tensor_scalar(base_i[:], rc[:], -float(S),
                                    float(C * P * S), Alu.mult, Alu.add)
            if USE_MAXIDX:
                # index within the winning chunk via the max_index unit
                gm8 = small_pool.tile([kh, 8], f32, tag=f"gm8{gi}")
                nc.vector.tensor_copy(gm8[:], gmax[:].broadcast_to((kh, 8)))
                idx8 = small_pool.tile([kh, 8], u32, tag=f"idx8{gi}")
                nc.vector.max_index(idx8[:], gm8[:], grow[:])
                flat_i = small_pool.tile([kh, 1], i32, tag=f"flat_i{gi}")
                nc.vector.tensor_tensor(flat_i[:], base_i[:],
                                        idx8[:, 0:1].bitcast(i32),
                                        op=Alu.add)
            else:
                cand_j = small_pool.tile([kh, S], f32, tag=f"cand_j{gi}")
                nc.vector.scalar_tensor_tensor(
                    cand_j[:], in0=grow[:], scalar=gmax[:],
                    in1=riota_j[0:kh, :], op0=Alu.is_ge, op1=Alu.mult)
                rj = small_pool.tile([kh, 1], f32, tag=f"rj{gi}")  # 1024 - j*
                nc.vector.reduce_max(rj[:], cand_j[:],
                                     axis=mybir.AxisListType.X)
                js = small_pool.tile([kh, 1], i32, tag=f"js{gi}")
                nc.vector.tensor_scalar(js[:], rj[:], -1.0, float(S),
                                        Alu.mult, Alu.add)
                flat_i = small_pool.tile([kh, 1], i32, tag=f"flat_i{gi}")
                nc.vector.tensor_tensor(flat_i[:], base_i[:], js[:],
                                        op=Alu.add)

            xx_i = small_pool.tile([kh, 1], i32, tag=f"xx_i{gi}")
            nc.vector.tensor_scalar(xx_i[:], flat_i[:], W - 1, None,
                                    Alu.bitwise_and)
            yy_i = small_pool.tile([kh, 1], i32, tag=f"yy_i{gi}")
            nc.vector.tensor_scalar(yy_i[:], flat_i[:], 9, None,
                                    Alu.logical_shift_right)

            valid = small_pool.tile([kh, 1], f32, tag=f"valid{gi}")
            nc.vector.tensor_scalar(valid[:], conf[:], 0.8, None, Alu.is_gt)

            # out = valid ? (4xx, 4yy, conf) : (-4, -4, -1)
            cand = small_pool.tile([kh, 3], f32, tag=f"cand{gi}")
            nc.vector.tensor_scalar(cand[:, 0:1], xx_i[:], 4.0, None, Alu.mult)
            nc.vector.tensor_scalar(cand[:, 1:2], yy_i[:], 4.0, None, Alu.mult)
            nc.vector.tensor_copy(cand[:, 2:3], conf[:])
            vb3 = small_pool.tile([kh, 3], i32, tag=f"vb3{gi}")
            nc.vector.tensor_scalar(vb3[:], cand[:], 0.0, valid[:],
                                    Alu.mult, Alu.add)
            out_sb = small_pool.tile([kh, 3], f32, tag=f"out_sb{gi}")
            nc.vector.memset(out_sb[:, 0:2], -4.0)
            nc.vector.memset(out_sb[:, 2:3], -1.0)
            nc.vector.copy_predicated(out_sb[:], vb3[:], cand[:])
            # early groups write out via gpsimd SWDGE so the scalar ring's
            # in-order program can't stall later groups' gathers behind them
            out_eng.dma_start(out_dram.ap()[k_lo:k_hi, :], out_sb[:])

        # Vector-order emission: each finale op's deps are ready by the
        # time Vector reaches it, so the engine never idles mid-stream.
        reduce_tile(0)
        reduce_tile(1)
        reduce_tile(2)
        finale_pre(0)
        reduce_tile(3)
        finale_post(0, nc.gpsimd)
        reduce_tile(4)
        finale_pre(1)
        finale_pre(2)
        reduce_tile(5)
        finale_post(1, nc.gpsimd)
        finale_post(2, nc.scalar)
        reduce_tile(6)
        finale_pre(3)
        finale_post(3, nc.scalar)

    nc.compile()
    return nc


def _get_nc():
    global _NC_CACHE
    if _NC_CACHE is None:
        _NC_CACHE = _build()
    return _NC_CACHE


def _shard(x: np.ndarray) -> list[dict[str, np.ndarray]]:
    xf = np.ascontiguousarray(np.asarray(x, dtype=np.float32).reshape(NK, ROW))
    shards = []
    for c in range(N_CORES):
        lo = c * K
        s = xf[lo:min(lo + K, NK)]
        if s.shape[0] < K:
            s = np.concatenate(
                [s, np.zeros((K - s.shape[0], ROW), np.float32)], axis=0)
        shards.append({"x": np.ascontiguousarray(s)})
    return shards


def _run(x, trace=False, **kw):
    nc = _get_nc()
    res = run_bass_kernel_spmd(nc, _shard(x), core_ids=list(range(N_CORES)),
                               trace=trace, **kw)
    out = np.concatenate([r["out"] for r in res.results], axis=0)[:NK]
    return out.astype(np.float32), res


def kernel(x: np.ndarray) -> np.ndarray:
    out, _ = _run(x, trace=False)
    return out
